# revision 21
# baseline (speedup 1.0000x reference)
import sys

sys.path.insert(0, "/opt/trn_rl_repo")

from collections import deque
from contextlib import ExitStack

import ml_dtypes
import numpy as np

from concourse import bass, mybir, tile
from concourse.bass_utils import run_bass_kernel_spmd
from concourse.vector_clock import ScopedClock


def _patched_drain_and_barrier(self, tick_clock, wait_clock):
    # Workaround: this compiler rejects a drain carrying >1 sem wait
    # ([NCC_INLA001]); split extra waits onto single-wait nops.
    drain_inst = self.nc.sync.drain()
    wait_clock.add_sem_waits(
        drain_inst.ins, ScopedClock({None: tick_clock.global_clock})
    )
    si = drain_inst.ins.sync_info
    waits = list(si.on_wait) if si and si.on_wait else []
    if len(waits) > 1:
        drain_inst.ins.sync_info = mybir.SyncInfo(
            on_wait=[waits[0]], on_update=list(si.on_update or [])
        )
        for w in waits[1:]:
            nop = self.nc.sync.nop(nofuse=True)
            nop.ins.sync_info = mybir.SyncInfo(on_wait=[w], on_update=[])
    self.nc.all_engine_barrier()
    popped = self.nc._tile_sem_poison_stack.pop()
    assert popped is self._sem_poison
    self.nc.clear_and_free_semaphores(list(self.sems.allocated().values()))
    self.nc.all_engine_barrier()


tile.TileContext._drain_and_barrier = _patched_drain_and_barrier


def _split_excess_waits(nc, limit=1):
    # Workaround: this compiler allows only one sem wait on several
    # instruction encodings; move extra waits onto same-engine nops.
    eng_map = {
        mybir.EngineType.PE: nc.tensor,
        mybir.EngineType.Activation: nc.scalar,
        mybir.EngineType.DVE: nc.vector,
        mybir.EngineType.Pool: nc.gpsimd,
        mybir.EngineType.SP: nc.sync,
    }
    for blk in nc.cur_f.blocks:
        orig = list(blk.instructions)
        out = []
        for ins in orig:
            si = ins.sync_info
            waits = list(si.on_wait) if si and si.on_wait else []
            eng = eng_map.get(ins.engine)
            if len(waits) > limit and eng is not None:
                extra, keep = waits[:-limit], waits[-limit:]
                for w in extra:
                    nop = eng.nop(nofuse=True).ins
                    tail = nc.cur_f.blocks[-1].instructions
                    assert tail[-1] is nop
                    tail.pop()
                    nop.sync_info = mybir.SyncInfo(on_wait=[w], on_update=[])
                    out.append(nop)
                ins.sync_info = mybir.SyncInfo(
                    on_wait=keep, on_update=list(si.on_update or [])
                )
            out.append(ins)
        blk.instructions[:] = out

bf16 = ml_dtypes.bfloat16
BF = bass.mybir.dt.bfloat16
F32 = bass.mybir.dt.float32
AF = mybir.ActivationFunctionType
ALU = mybir.AluOpType

B, S, E, H, D = 2, 2048, 2048, 16, 128
BS = B * S
NCORES = 8
HPC = H // NCORES  # heads per core
DC = HPC * D  # per-core head-dim width (256)
SCALE = 1.0 / float(np.sqrt(D))

TRACE = False
LAST_RESULTS = None
_NC_CACHE = None


def _build():
    nc = bass.Bass()
    # host-repacked layouts: first dim is the SBUF partition, so every
    # load is one (or few) big contiguous-ish DMA — SP issue time at
    # ~620ns/dma_start was a measured bottleneck.
    xh = nc.declare_dram_parameter("xh", (128, 16, BS), BF, isOutput=False)
    wqh = nc.declare_dram_parameter("wqh", (128, 16, DC), BF, isOutput=False)
    wkh = nc.declare_dram_parameter("wkh", (128, 16, DC), BF, isOutput=False)
    wvh = nc.declare_dram_parameter("wvh", (128, 16, DC), BF, isOutput=False)
    woh = nc.declare_dram_parameter("woh", (128, HPC, E), BF, isOutput=False)
    bqh = nc.declare_dram_parameter("bq", (128, HPC), F32, isOutput=False)
    bkh = nc.declare_dram_parameter("bk", (128, HPC), F32, isOutput=False)
    mskh = nc.declare_dram_parameter("mask", (128, 4, 512), BF, isOutput=False)
    onkd = nc.declare_dram_parameter("onesk", (128, 128), BF, isOutput=False)
    yd = nc.declare_dram_parameter("y", (BS, E), BF, isOutput=True)

    with ExitStack() as ctx:
        tc = ctx.enter_context(tile.TileContext(nc))
        wp = ctx.enter_context(tc.tile_pool(name="wp", bufs=1))
        bp = ctx.enter_context(tc.tile_pool(name="bp", bufs=1))
        pp = ctx.enter_context(tc.tile_pool(name="pp", bufs=17))
        dp = ctx.enter_context(tc.tile_pool(name="dp", bufs=2))
        yp = ctx.enter_context(tc.tile_pool(name="yp", bufs=3))
        ps = ctx.enter_context(tc.tile_pool(name="ps", bufs=1, space="PSUM"))

        wq_sb = wp.tile([128, 16, DC], BF)
        wk_sb = wp.tile([128, 16, DC], BF)
        wv_sb = wp.tile([128, 16, DC], BF)
        wo_sb = wp.tile([128, HPC, E], BF)
        bq_sb = wp.tile([128, HPC], F32)
        bk_sb = wp.tile([128, HPC], F32)
        msk_sb = wp.tile([128, 4, 512], BF)
        onk_sb = wp.tile([128, 128], BF)

        # --- initial DMA: small/bias first, then weights + x(b0) j-major
        x0_sb = bp.tile([128, 16, S], BF, tag="x", bufs=1)
        for i in range(4):
            nc.sync.dma_start(wq_sb[:, 4 * i : 4 * i + 4, :], wqh[:, 4 * i : 4 * i + 4, :])
            nc.sync.dma_start(
                x0_sb[:, 4 * i : 4 * i + 4, 0:512], xh[:, 4 * i : 4 * i + 4, 0:512]
            )
        for i in range(4):
            nc.sync.dma_start(wk_sb[:, 4 * i : 4 * i + 4, :], wkh[:, 4 * i : 4 * i + 4, :])
        nc.sync.dma_start(bq_sb[:], bqh[:])
        nc.sync.dma_start(bk_sb[:], bkh[:])
        nc.sync.dma_start(onk_sb[:], onkd[:])
        for j in range(1, 4):
            js = slice(j * 512, (j + 1) * 512)
            nc.sync.dma_start(x0_sb[:, 0:8, js], xh[:, 0:8, js])
            nc.sync.dma_start(x0_sb[:, 8:16, js], xh[:, 8:16, js])
        nc.sync.dma_start(msk_sb[:], mskh[:])
        for i in range(4):
            nc.sync.dma_start(wv_sb[:, 4 * i : 4 * i + 4, :], wvh[:, 4 * i : 4 * i + 4, :])
        nc.sync.dma_start(wo_sb[:], woh[:])

        def emit_qkv(x_sb, qT_sb, kT_sb, v_sb):
            # q/k projections (contract E in 16 chunks of 128)
            for m in range(HPC):
                for j in range(4):
                    js = slice(j * 512, (j + 1) * 512)
                    q_ps = ps.tile([128, 512], F32, tag="pr", bufs=2)
                    for t in range(16):
                        nc.tensor.matmul(
                            q_ps[:],
                            wq_sb[:, t, m * 128 : (m + 1) * 128],
                            x_sb[:, t, js],
                            start=(t == 0),
                            stop=(t == 15),
                        )
                    nc.scalar.activation(
                        qT_sb[:, m, js], q_ps[:], AF.Identity, bias=bq_sb[:, m : m + 1]
                    )
                    k_ps = ps.tile([128, 512], F32, tag="pr", bufs=2)
                    for t in range(16):
                        nc.tensor.matmul(
                            k_ps[:],
                            wk_sb[:, t, m * 128 : (m + 1) * 128],
                            x_sb[:, t, js],
                            start=(t == 0),
                            stop=(t == 15),
                        )
                    nc.scalar.activation(
                        kT_sb[:, m, js], k_ps[:], AF.Identity, bias=bk_sb[:, m : m + 1]
                    )
            for si in range(16):
                v_ps = ps.tile([128, DC], F32, tag="pr", bufs=2)
                for t in range(16):
                    nc.tensor.matmul(
                        v_ps[:],
                        x_sb[:, t, si * 128 : (si + 1) * 128],
                        wv_sb[:, t, :],
                        start=(t == 0),
                        stop=(t == 15),
                    )
                nc.vector.tensor_copy(v_sb[:, si, :], v_ps[:])

        def emit_attn(qT_sb, kT_sb, v_sb, ctxN_sb, h, qb, filler=None):
            # causal attention, scores kept transposed [k, q]
            hd = slice(h * 128, (h + 1) * 128)
            kmax = 4 * qb + 4
            pts = []
            den_ps = ps.tile([128, 512], F32, tag="dn", bufs=1)
            ctx_ps = ps.tile([128, 512], F32, tag="cx", bufs=2)
            LAG = 3
            # interleave den/ctx accumulation (lagging LAG tiles)
            # between score matmuls so PE never waits on ACT exp.
            # Diagonal k-tiles only touch q >= k (qoff), skipping the
            # dead upper-triangle work on PE and ACT.
            for kc in range(kmax + LAG):
                if kc < kmax:
                    sc_ps = ps.tile([128, 512], F32, tag="sc", bufs=3)
                    diag = kc - 4 * qb
                    qoff = max(diag, 0) * 128
                    nc.tensor.matmul(
                        sc_ps[:, qoff:512],
                        kT_sb[:, h, kc * 128 : (kc + 1) * 128],
                        qT_sb[:, h, qb * 512 + qoff : (qb + 1) * 512],
                        start=True,
                        stop=True,
                    )
                    p_t = pp.tile([128, 512], BF)
                    nc.scalar.activation(
                        p_t[:, qoff:512], sc_ps[:, qoff:512], AF.Exp
                    )
                    if diag >= 0:
                        nc.vector.tensor_tensor(
                            p_t[:, qoff:512],
                            p_t[:, qoff:512],
                            msk_sb[:, diag, qoff:512],
                            ALU.mult,
                        )
                    pts.append((p_t, qoff))
                j = kc - LAG
                if j >= 0:
                    p_j, qo = pts[j]
                    nc.tensor.matmul(
                        den_ps[:, qo:512],
                        onk_sb[:],
                        p_j[:, qo:512],
                        start=(j == 0),
                        stop=(j == kmax - 1),
                    )
                    nc.tensor.matmul(
                        ctx_ps[:, qo:512],
                        v_sb[:, j, hd],
                        p_j[:, qo:512],
                        start=(j == 0),
                        stop=(j == kmax - 1),
                    )
                # independent out-proj filler keeps PE busy when ACT's
                # exp pipeline is the local pacer
                if filler:
                    filler.popleft()()
            lnd_sb = dp.tile([128, 512], F32, tag="lnd", bufs=2)
            nc.scalar.activation(lnd_sb[:], den_ps[:], AF.Ln)
            recb_sb = dp.tile([128, 512], F32, tag="recb", bufs=2)
            nc.scalar.activation(recb_sb[:], lnd_sb[:], AF.Exp, scale=-1.0)
            nc.vector.tensor_tensor(
                ctxN_sb[:, h, qb * 512 : (qb + 1) * 512],
                ctx_ps[:],
                recb_sb[:],
                ALU.mult,
            )

        def outproj_thunks(ctxN_sb, s0, qb, split=False):
            # output projection rows qb*512..qb*512+511 (contract d=256)
            # as a deque of (qc, eb) pair-thunks so they can be spliced
            # into the attention loop as independent PE filler work.
            # y evacuated mostly via DVE, one [128, 2048] store per block.
            thunks = deque()
            for qc in range(qb * 4, qb * 4 + 4):
                y_box = []
                for eb in range(4):
                    def pair(qc=qc, eb=eb, y_box=y_box):
                        if eb == 0:
                            y_box.append(
                                yp.tile([128, E], BF, tag="y_t", bufs=3, name="y_t")
                            )
                        y_t = y_box[0]
                        y_ps = ps.tile([128, 512], F32, tag="pr", bufs=2)
                        nc.tensor.matmul(
                            y_ps[:],
                            ctxN_sb[:, 0, qc * 128 : (qc + 1) * 128],
                            wo_sb[:, 0, eb * 512 : (eb + 1) * 512],
                            start=True,
                            stop=False,
                        )
                        nc.tensor.matmul(
                            y_ps[:],
                            ctxN_sb[:, 1, qc * 128 : (qc + 1) * 128],
                            wo_sb[:, 1, eb * 512 : (eb + 1) * 512],
                            start=False,
                            stop=True,
                        )
                        if split and eb % 2 == 1:
                            nc.scalar.copy(
                                y_t[:, eb * 512 : (eb + 1) * 512], y_ps[:]
                            )
                        else:
                            nc.vector.tensor_copy(
                                y_t[:, eb * 512 : (eb + 1) * 512], y_ps[:]
                            )
                        if eb == 3:
                            nc.sync.dma_start(
                                yd[s0 + qc * 128 : s0 + (qc + 1) * 128, :],
                                y_t[:],
                            )
                    thunks.append(pair)
            return thunks

        q0T = bp.tile([128, HPC, S], BF, tag="qT", bufs=1)
        k0T = bp.tile([128, HPC, S], BF, tag="kT", bufs=1)
        v0 = bp.tile([128, 16, DC], BF, tag="v", bufs=1)
        c0 = bp.tile([128, HPC, S], BF, tag="ctxN", bufs=2)
        emit_qkv(x0_sb, q0T, k0T, v0)

        # batch-1 x loads are issued during batch-0 attention: the x slot
        # is WAR-free right when attention starts, and the loads don't
        # queue behind y(b0) stores.
        x1_sb = bp.tile([128, 16, S], BF, tag="x", bufs=1)
        for qb in range(4):
            js = slice(qb * 512, (qb + 1) * 512)
            nc.sync.dma_start(x1_sb[:, 0:8, js], xh[:, 0:8, S + qb * 512 : S + (qb + 1) * 512])
            nc.sync.dma_start(x1_sb[:, 8:16, js], xh[:, 8:16, S + qb * 512 : S + (qb + 1) * 512])
        pend = deque()
        for qb in range(4):
            filler = pend.popleft() if pend else deque()
            emit_attn(q0T, k0T, v0, c0, 0, qb, filler)
            emit_attn(q0T, k0T, v0, c0, 1, qb, filler)
            while filler:
                filler.popleft()()
            pend.append(outproj_thunks(c0, 0, qb))

        q1T = bp.tile([128, HPC, S], BF, tag="qT", bufs=1)
        k1T = bp.tile([128, HPC, S], BF, tag="kT", bufs=1)
        v1 = bp.tile([128, 16, DC], BF, tag="v", bufs=1)
        c1 = bp.tile([128, HPC, S], BF, tag="ctxN", bufs=2)
        emit_qkv(x1_sb, q1T, k1T, v1)

        for qb in range(4):
            filler = pend.popleft()
            emit_attn(q1T, k1T, v1, c1, 0, qb, filler)
            emit_attn(q1T, k1T, v1, c1, 1, qb, filler)
            while filler:
                filler.popleft()()
            pend.append(outproj_thunks(c1, S, qb, split=(qb == 3)))
        last = pend.popleft()
        while last:
            last.popleft()()
    _split_excess_waits(nc)
    return nc


def _mask_np():
    m = np.zeros((4, 128, 512), np.float32)
    kk = np.arange(128)[:, None]
    qq = np.arange(512)[None, :]
    for r in range(4):
        m[r] = np.where(kk + 128 * r > qq, 0.0, 1.0)
    return np.ascontiguousarray(m.transpose(1, 0, 2)).astype(bf16)  # [128, 4, 512]


def _pack_w(wT):
    # [E, DC] (contract-major) -> [128, 16, DC] partition-major
    return np.ascontiguousarray(wT.reshape(16, 128, -1).transpose(1, 0, 2))


def kernel(**inputs):
    global LAST_RESULTS, _NC_CACHE
    x = np.asarray(inputs["x"], np.float32)
    Wq = np.asarray(inputs["Wq"], np.float32)
    bq = np.asarray(inputs["bq"], np.float32)
    Wk = np.asarray(inputs["Wk"], np.float32)
    bk = np.asarray(inputs["bk"], np.float32)
    Wv = np.asarray(inputs["Wv"], np.float32)
    bv = np.asarray(inputs["bv"], np.float32)
    Wo = np.asarray(inputs["Wo"], np.float32)
    bo = np.asarray(inputs["bo"], np.float32)

    xT = x.reshape(BS, E).T.astype(bf16)  # [E, BS]
    xhp = np.ascontiguousarray(xT.reshape(16, 128, BS).transpose(1, 0, 2))
    mask = _mask_np()
    onesk = np.ones((128, 128), bf16)

    in_maps = []
    for c in range(NCORES):
        dsl = slice(c * DC, (c + 1) * DC)
        in_maps.append(
            {
                "xh": xhp,
                "wqh": _pack_w((Wq[dsl].T * SCALE).astype(bf16)),
                "wkh": _pack_w(Wk[dsl].T.astype(bf16)),
                "wvh": _pack_w(Wv[dsl].T.astype(bf16)),
                "woh": np.ascontiguousarray(
                    Wo[:, dsl].T.astype(bf16).reshape(HPC, 128, E).transpose(1, 0, 2)
                ),
                "bq": np.ascontiguousarray(
                    (bq[dsl] * SCALE).astype(np.float32).reshape(HPC, 128).T
                ),
                "bk": np.ascontiguousarray(
                    bk[dsl].astype(np.float32).reshape(HPC, 128).T
                ),
                "mask": mask,
                "onesk": onesk,
            }
        )

    if _NC_CACHE is None:
        _NC_CACHE = _build()
    res = run_bass_kernel_spmd(_NC_CACHE, in_maps, core_ids=list(range(NCORES)), trace=TRACE)
    LAST_RESULTS = res

    acc = None
    for r in res.results:
        yc = np.asarray(r["y"]).astype(np.float32)
        acc = yc if acc is None else acc + yc
    bo_eff = bo + bv @ Wo.T
    acc += bo_eff[None, :]
    return acc.reshape(B, S, E).astype(np.float32)


# revision 22
# speedup vs baseline: 1.0108x; 1.0108x over previous
import sys

sys.path.insert(0, "/opt/trn_rl_repo")

from collections import deque
from contextlib import ExitStack

import ml_dtypes
import numpy as np

from concourse import bass, mybir, tile
from concourse.bass_utils import run_bass_kernel_spmd
from concourse.vector_clock import ScopedClock


def _patched_drain_and_barrier(self, tick_clock, wait_clock):
    # Workaround: this compiler rejects a drain carrying >1 sem wait
    # ([NCC_INLA001]); split extra waits onto single-wait nops.
    drain_inst = self.nc.sync.drain()
    wait_clock.add_sem_waits(
        drain_inst.ins, ScopedClock({None: tick_clock.global_clock})
    )
    si = drain_inst.ins.sync_info
    waits = list(si.on_wait) if si and si.on_wait else []
    if len(waits) > 1:
        drain_inst.ins.sync_info = mybir.SyncInfo(
            on_wait=[waits[0]], on_update=list(si.on_update or [])
        )
        for w in waits[1:]:
            nop = self.nc.sync.nop(nofuse=True)
            nop.ins.sync_info = mybir.SyncInfo(on_wait=[w], on_update=[])
    self.nc.all_engine_barrier()
    popped = self.nc._tile_sem_poison_stack.pop()
    assert popped is self._sem_poison
    self.nc.clear_and_free_semaphores(list(self.sems.allocated().values()))
    self.nc.all_engine_barrier()


tile.TileContext._drain_and_barrier = _patched_drain_and_barrier


def _split_excess_waits(nc, limit=1):
    # Workaround: this compiler allows only one sem wait on several
    # instruction encodings; move extra waits onto same-engine nops.
    eng_map = {
        mybir.EngineType.PE: nc.tensor,
        mybir.EngineType.Activation: nc.scalar,
        mybir.EngineType.DVE: nc.vector,
        mybir.EngineType.Pool: nc.gpsimd,
        mybir.EngineType.SP: nc.sync,
    }
    for blk in nc.cur_f.blocks:
        orig = list(blk.instructions)
        out = []
        for ins in orig:
            si = ins.sync_info
            waits = list(si.on_wait) if si and si.on_wait else []
            eng = eng_map.get(ins.engine)
            if len(waits) > limit and eng is not None:
                extra, keep = waits[:-limit], waits[-limit:]
                for w in extra:
                    nop = eng.nop(nofuse=True).ins
                    tail = nc.cur_f.blocks[-1].instructions
                    assert tail[-1] is nop
                    tail.pop()
                    nop.sync_info = mybir.SyncInfo(on_wait=[w], on_update=[])
                    out.append(nop)
                ins.sync_info = mybir.SyncInfo(
                    on_wait=keep, on_update=list(si.on_update or [])
                )
            out.append(ins)
        blk.instructions[:] = out

bf16 = ml_dtypes.bfloat16
BF = bass.mybir.dt.bfloat16
F32 = bass.mybir.dt.float32
AF = mybir.ActivationFunctionType
ALU = mybir.AluOpType

B, S, E, H, D = 2, 2048, 2048, 16, 128
BS = B * S
NCORES = 8
HPC = H // NCORES  # heads per core
DC = HPC * D  # per-core head-dim width (256)
SCALE = 1.0 / float(np.sqrt(D))

TRACE = False
LAST_RESULTS = None
_NC_CACHE = None


def _build():
    nc = bass.Bass()
    # host-repacked layouts: first dim is the SBUF partition, so every
    # load is one (or few) big contiguous-ish DMA — SP issue time at
    # ~620ns/dma_start was a measured bottleneck.
    xh = nc.declare_dram_parameter("xh", (128, 16, BS), BF, isOutput=False)
    wqh = nc.declare_dram_parameter("wqh", (128, 16, DC), BF, isOutput=False)
    wkh = nc.declare_dram_parameter("wkh", (128, 16, DC), BF, isOutput=False)
    wvh = nc.declare_dram_parameter("wvh", (128, 16, DC), BF, isOutput=False)
    woh = nc.declare_dram_parameter("woh", (128, HPC, E), BF, isOutput=False)
    bqh = nc.declare_dram_parameter("bq", (128, HPC), F32, isOutput=False)
    bkh = nc.declare_dram_parameter("bk", (128, HPC), F32, isOutput=False)
    mskh = nc.declare_dram_parameter("mask", (128, 4, 512), BF, isOutput=False)
    onkd = nc.declare_dram_parameter("onesk", (128, 128), BF, isOutput=False)
    yd = nc.declare_dram_parameter("y", (BS, E), BF, isOutput=True)

    with ExitStack() as ctx:
        tc = ctx.enter_context(tile.TileContext(nc))
        wp = ctx.enter_context(tc.tile_pool(name="wp", bufs=1))
        bp = ctx.enter_context(tc.tile_pool(name="bp", bufs=1))
        pp = ctx.enter_context(tc.tile_pool(name="pp", bufs=17))
        dp = ctx.enter_context(tc.tile_pool(name="dp", bufs=2))
        yp = ctx.enter_context(tc.tile_pool(name="yp", bufs=3))
        ps = ctx.enter_context(tc.tile_pool(name="ps", bufs=1, space="PSUM"))

        wq_sb = wp.tile([128, 16, DC], BF)
        wk_sb = wp.tile([128, 16, DC], BF)
        wv_sb = wp.tile([128, 16, DC], BF)
        wo_sb = wp.tile([128, HPC, E], BF)
        bq_sb = wp.tile([128, HPC], F32)
        bk_sb = wp.tile([128, HPC], F32)
        msk_sb = wp.tile([128, 4, 512], BF)
        onk_sb = wp.tile([128, 128], BF)

        # --- initial DMA: small/bias first, then weights + x(b0) j-major
        x0_sb = bp.tile([128, 16, S], BF, tag="x", bufs=1)
        for i in range(4):
            nc.sync.dma_start(wq_sb[:, 4 * i : 4 * i + 4, :], wqh[:, 4 * i : 4 * i + 4, :])
            nc.sync.dma_start(wk_sb[:, 4 * i : 4 * i + 4, :], wkh[:, 4 * i : 4 * i + 4, :])
            nc.sync.dma_start(
                x0_sb[:, 4 * i : 4 * i + 4, 0:512], xh[:, 4 * i : 4 * i + 4, 0:512]
            )
        nc.sync.dma_start(bq_sb[:], bqh[:])
        nc.sync.dma_start(bk_sb[:], bkh[:])
        nc.sync.dma_start(onk_sb[:], onkd[:])
        for j in range(1, 4):
            js = slice(j * 512, (j + 1) * 512)
            nc.sync.dma_start(x0_sb[:, 0:8, js], xh[:, 0:8, js])
            nc.sync.dma_start(x0_sb[:, 8:16, js], xh[:, 8:16, js])
        nc.sync.dma_start(msk_sb[:], mskh[:])
        for i in range(4):
            nc.sync.dma_start(wv_sb[:, 4 * i : 4 * i + 4, :], wvh[:, 4 * i : 4 * i + 4, :])
        nc.sync.dma_start(wo_sb[:], woh[:])

        def emit_qkv(x_sb, qT_sb, kT_sb, v_sb):
            # q/k projections (contract E in 16 chunks of 128)
            for m in range(HPC):
                for j in range(4):
                    js = slice(j * 512, (j + 1) * 512)
                    q_ps = ps.tile([128, 512], F32, tag="pr", bufs=2)
                    for t in range(16):
                        nc.tensor.matmul(
                            q_ps[:],
                            wq_sb[:, t, m * 128 : (m + 1) * 128],
                            x_sb[:, t, js],
                            start=(t == 0),
                            stop=(t == 15),
                        )
                    nc.scalar.activation(
                        qT_sb[:, m, js], q_ps[:], AF.Identity, bias=bq_sb[:, m : m + 1]
                    )
                    k_ps = ps.tile([128, 512], F32, tag="pr", bufs=2)
                    for t in range(16):
                        nc.tensor.matmul(
                            k_ps[:],
                            wk_sb[:, t, m * 128 : (m + 1) * 128],
                            x_sb[:, t, js],
                            start=(t == 0),
                            stop=(t == 15),
                        )
                    nc.scalar.activation(
                        kT_sb[:, m, js], k_ps[:], AF.Identity, bias=bk_sb[:, m : m + 1]
                    )
            for si in range(16):
                v_ps = ps.tile([128, DC], F32, tag="pr", bufs=2)
                for t in range(16):
                    nc.tensor.matmul(
                        v_ps[:],
                        x_sb[:, t, si * 128 : (si + 1) * 128],
                        wv_sb[:, t, :],
                        start=(t == 0),
                        stop=(t == 15),
                    )
                nc.vector.tensor_copy(v_sb[:, si, :], v_ps[:])

        def emit_attn(qT_sb, kT_sb, v_sb, ctxN_sb, h, qb, filler=None):
            # causal attention, scores kept transposed [k, q]
            hd = slice(h * 128, (h + 1) * 128)
            kmax = 4 * qb + 4
            pts = []
            den_ps = ps.tile([128, 512], F32, tag="dn", bufs=1)
            ctx_ps = ps.tile([128, 512], F32, tag="cx", bufs=2)
            LAG = 3
            # interleave den/ctx accumulation (lagging LAG tiles)
            # between score matmuls so PE never waits on ACT exp.
            # Diagonal k-tiles only touch q >= k (qoff), skipping the
            # dead upper-triangle work on PE and ACT.
            for kc in range(kmax + LAG):
                if kc < kmax:
                    sc_ps = ps.tile([128, 512], F32, tag="sc", bufs=3)
                    diag = kc - 4 * qb
                    qoff = max(diag, 0) * 128
                    nc.tensor.matmul(
                        sc_ps[:, qoff:512],
                        kT_sb[:, h, kc * 128 : (kc + 1) * 128],
                        qT_sb[:, h, qb * 512 + qoff : (qb + 1) * 512],
                        start=True,
                        stop=True,
                    )
                    p_t = pp.tile([128, 512], BF)
                    nc.scalar.activation(
                        p_t[:, qoff:512], sc_ps[:, qoff:512], AF.Exp
                    )
                    if diag >= 0:
                        nc.vector.tensor_tensor(
                            p_t[:, qoff:512],
                            p_t[:, qoff:512],
                            msk_sb[:, diag, qoff:512],
                            ALU.mult,
                        )
                    pts.append((p_t, qoff))
                j = kc - LAG
                if j >= 0:
                    p_j, qo = pts[j]
                    nc.tensor.matmul(
                        den_ps[:, qo:512],
                        onk_sb[:],
                        p_j[:, qo:512],
                        start=(j == 0),
                        stop=(j == kmax - 1),
                    )
                    nc.tensor.matmul(
                        ctx_ps[:, qo:512],
                        v_sb[:, j, hd],
                        p_j[:, qo:512],
                        start=(j == 0),
                        stop=(j == kmax - 1),
                    )
                # independent out-proj filler keeps PE busy when ACT's
                # exp pipeline is the local pacer
                if filler:
                    filler.popleft()()
            lnd_sb = dp.tile([128, 512], F32, tag="lnd", bufs=2)
            nc.scalar.activation(lnd_sb[:], den_ps[:], AF.Ln)
            recb_sb = dp.tile([128, 512], F32, tag="recb", bufs=2)
            nc.scalar.activation(recb_sb[:], lnd_sb[:], AF.Exp, scale=-1.0)
            nc.vector.tensor_tensor(
                ctxN_sb[:, h, qb * 512 : (qb + 1) * 512],
                ctx_ps[:],
                recb_sb[:],
                ALU.mult,
            )

        def outproj_thunks(ctxN_sb, s0, qb, split=False):
            # output projection rows qb*512..qb*512+511 (contract d=256)
            # as a deque of (qc, eb) pair-thunks so they can be spliced
            # into the attention loop as independent PE filler work.
            # y evacuated mostly via DVE, one [128, 2048] store per block.
            thunks = deque()
            for qc in range(qb * 4, qb * 4 + 4):
                y_box = []
                for eb in range(4):
                    def pair(qc=qc, eb=eb, y_box=y_box):
                        if eb == 0:
                            y_box.append(
                                yp.tile([128, E], BF, tag="y_t", bufs=3, name="y_t")
                            )
                        y_t = y_box[0]
                        y_ps = ps.tile([128, 512], F32, tag="pr", bufs=2)
                        nc.tensor.matmul(
                            y_ps[:],
                            ctxN_sb[:, 0, qc * 128 : (qc + 1) * 128],
                            wo_sb[:, 0, eb * 512 : (eb + 1) * 512],
                            start=True,
                            stop=False,
                        )
                        nc.tensor.matmul(
                            y_ps[:],
                            ctxN_sb[:, 1, qc * 128 : (qc + 1) * 128],
                            wo_sb[:, 1, eb * 512 : (eb + 1) * 512],
                            start=False,
                            stop=True,
                        )
                        if split and eb % 2 == 1:
                            nc.scalar.copy(
                                y_t[:, eb * 512 : (eb + 1) * 512], y_ps[:]
                            )
                        else:
                            nc.vector.tensor_copy(
                                y_t[:, eb * 512 : (eb + 1) * 512], y_ps[:]
                            )
                        if eb == 3:
                            nc.sync.dma_start(
                                yd[s0 + qc * 128 : s0 + (qc + 1) * 128, :],
                                y_t[:],
                            )
                    thunks.append(pair)
            return thunks

        q0T = bp.tile([128, HPC, S], BF, tag="qT", bufs=1)
        k0T = bp.tile([128, HPC, S], BF, tag="kT", bufs=1)
        v0 = bp.tile([128, 16, DC], BF, tag="v", bufs=1)
        c0 = bp.tile([128, HPC, S], BF, tag="ctxN", bufs=2)
        emit_qkv(x0_sb, q0T, k0T, v0)

        # batch-1 x loads are issued during batch-0 attention: the x slot
        # is WAR-free right when attention starts, and the loads don't
        # queue behind y(b0) stores.
        x1_sb = bp.tile([128, 16, S], BF, tag="x", bufs=1)
        for qb in range(4):
            js = slice(qb * 512, (qb + 1) * 512)
            nc.sync.dma_start(x1_sb[:, 0:8, js], xh[:, 0:8, S + qb * 512 : S + (qb + 1) * 512])
            nc.sync.dma_start(x1_sb[:, 8:16, js], xh[:, 8:16, S + qb * 512 : S + (qb + 1) * 512])
        pend = deque()
        for qb in range(4):
            filler = pend.popleft() if pend else deque()
            emit_attn(q0T, k0T, v0, c0, 0, qb, filler)
            emit_attn(q0T, k0T, v0, c0, 1, qb, filler)
            while filler:
                filler.popleft()()
            pend.append(outproj_thunks(c0, 0, qb))

        q1T = bp.tile([128, HPC, S], BF, tag="qT", bufs=1)
        k1T = bp.tile([128, HPC, S], BF, tag="kT", bufs=1)
        v1 = bp.tile([128, 16, DC], BF, tag="v", bufs=1)
        c1 = bp.tile([128, HPC, S], BF, tag="ctxN", bufs=2)
        emit_qkv(x1_sb, q1T, k1T, v1)

        for qb in range(4):
            filler = pend.popleft()
            emit_attn(q1T, k1T, v1, c1, 0, qb, filler)
            emit_attn(q1T, k1T, v1, c1, 1, qb, filler)
            while filler:
                filler.popleft()()
            pend.append(outproj_thunks(c1, S, qb, split=(qb == 3)))
        last = pend.popleft()
        while last:
            last.popleft()()
    _split_excess_waits(nc)
    return nc


def _mask_np():
    m = np.zeros((4, 128, 512), np.float32)
    kk = np.arange(128)[:, None]
    qq = np.arange(512)[None, :]
    for r in range(4):
        m[r] = np.where(kk + 128 * r > qq, 0.0, 1.0)
    return np.ascontiguousarray(m.transpose(1, 0, 2)).astype(bf16)  # [128, 4, 512]


def _pack_w(wT):
    # [E, DC] (contract-major) -> [128, 16, DC] partition-major
    return np.ascontiguousarray(wT.reshape(16, 128, -1).transpose(1, 0, 2))


def kernel(**inputs):
    global LAST_RESULTS, _NC_CACHE
    x = np.asarray(inputs["x"], np.float32)
    Wq = np.asarray(inputs["Wq"], np.float32)
    bq = np.asarray(inputs["bq"], np.float32)
    Wk = np.asarray(inputs["Wk"], np.float32)
    bk = np.asarray(inputs["bk"], np.float32)
    Wv = np.asarray(inputs["Wv"], np.float32)
    bv = np.asarray(inputs["bv"], np.float32)
    Wo = np.asarray(inputs["Wo"], np.float32)
    bo = np.asarray(inputs["bo"], np.float32)

    xT = x.reshape(BS, E).T.astype(bf16)  # [E, BS]
    xhp = np.ascontiguousarray(xT.reshape(16, 128, BS).transpose(1, 0, 2))
    mask = _mask_np()
    onesk = np.ones((128, 128), bf16)

    in_maps = []
    for c in range(NCORES):
        dsl = slice(c * DC, (c + 1) * DC)
        in_maps.append(
            {
                "xh": xhp,
                "wqh": _pack_w((Wq[dsl].T * SCALE).astype(bf16)),
                "wkh": _pack_w(Wk[dsl].T.astype(bf16)),
                "wvh": _pack_w(Wv[dsl].T.astype(bf16)),
                "woh": np.ascontiguousarray(
                    Wo[:, dsl].T.astype(bf16).reshape(HPC, 128, E).transpose(1, 0, 2)
                ),
                "bq": np.ascontiguousarray(
                    (bq[dsl] * SCALE).astype(np.float32).reshape(HPC, 128).T
                ),
                "bk": np.ascontiguousarray(
                    bk[dsl].astype(np.float32).reshape(HPC, 128).T
                ),
                "mask": mask,
                "onesk": onesk,
            }
        )

    if _NC_CACHE is None:
        _NC_CACHE = _build()
    res = run_bass_kernel_spmd(_NC_CACHE, in_maps, core_ids=list(range(NCORES)), trace=TRACE)
    LAST_RESULTS = res

    acc = None
    for r in res.results:
        yc = np.asarray(r["y"]).astype(np.float32)
        acc = yc if acc is None else acc + yc
    bo_eff = bo + bv @ Wo.T
    acc += bo_eff[None, :]
    return acc.reshape(B, S, E).astype(np.float32)
